# revision 12
# baseline (speedup 1.0000x reference)
"""GQA kernel for trn2, 8 NeuronCores — v2 (ACT-bound pipeline).

Problem: B=1, S=2048, D=128, H=32, KVH=8, REP=4, rope(theta=1e4) on k AND v,
softmax(q@k^T/sqrt(128)) @ v, out @ Wo + bo.  The reference replicates torch
.view() semantics: (B,S,H*D) -> (B,H,S,D) is a FLAT reinterpretation, so
q-head h is rows [h*64,(h+1)*64) of the projection output reinterpreted as
(2048,128), and kv-head g is rows [g*256,(g+1)*256) of the k/v projections.

Sharding: core c owns kv-head g=c and q-heads {c, c+8, c+16, c+24}.
Storage order: q-position j_q = b*64+a <-> s' = 32a+b; kv j_k = b*256+a <->
t = 8a+b.  Host permutes rope tables into storage order, unpermutes output
rows, and sums per-core partials (Wo is a per-head row-block contraction).

v2 engine plan (per core) — the Activation engine (exp: 1 elem/cycle/
partition over 16.8M probs => ~127us busy) is the pinned bottleneck;
everything else hides under it:
 - scores: fp8e4 DoubleRow matmuls, d contracted as 2x64 partitions
   (q,k scaled x4 into fp8 range; the x4 folds into Wq / K-rope tables)
 - exp: ACT psum->sbuf bf16 probs, jk-tile groups {3,3,3,3,2,2} per
   (chunk, head) step; score psum = 2 x [128,1536] double-buffered
 - AV: bf16 V rows (lhsT) x bf16 probs, psum accumulation over 16 jk tiles
 - dn: gpsimd cast-DMA makes an fp8 copy of probs; fp8 ones x fp8 probs
   DoubleRow matmuls (half-rate rows) accumulate the denominators
 - normalize: DVE reciprocal + tensor_tensor -> bf16 OHT
 - Wo: bf16 matmuls accumulating 4 heads; DVE copy -> f32 out; out DMA
PE ~105us < ACT ~127us; DVE ~50us (K rope + drains + normalize);
Pool/gpsimd does the V rope + the pr8 cast-DMA queue.
"""

import sys

sys.path.insert(0, "/opt/trn_rl_repo")

import numpy as np
import ml_dtypes

import concourse.bass as bass
import concourse.mybir as mybir
import concourse.tile as tile
from concourse import bacc
from concourse.bass_utils import run_bass_kernel_spmd

F32 = mybir.dt.float32
F32R = mybir.dt.float32r
BF16 = mybir.dt.bfloat16
FP8 = mybir.dt.float8e4
DR = mybir.MatmulPerfMode.DoubleRow

B, S, D = 1, 2048, 128
H, KVH, REP = 32, 8, 4
NCORES = 8
SCALE = 1.0 / np.sqrt(128.0)
QK_SCALE = 4.0
ROPE_THETA = 10000.0

NP_BF16 = ml_dtypes.bfloat16
NP_FP8 = np.dtype(mybir.dt.np(FP8))

_j = np.arange(S)
PERM_Q = 32 * (_j % 64) + _j // 64
PERM_K = 8 * (_j % 256) + _j // 256

GROUPS = [(0, 3), (3, 6), (6, 9), (9, 12), (12, 14), (14, 16)]

_nc_cache = {}


def _rope_tables():
    inv_freq = 1.0 / (ROPE_THETA ** (np.arange(0, D, 2, dtype=np.float64) / D))
    ang = np.arange(S, dtype=np.float64)[:, None] * inv_freq  # (S, 64)
    cos = np.cos(ang)
    sin = np.sin(ang)

    # K tables in [d, j_k] layout, 128 partitions: cos duplicated across the
    # two d halves, sin with the rotate-half sign folded ([-sin; +sin]).
    # fp8 x4 pre-scale folded in.
    t = PERM_K
    c64 = cos[t, :].T * QK_SCALE   # (64, S)
    s64 = sin[t, :].T * QK_SCALE
    cosK = np.ascontiguousarray(np.concatenate([c64, c64], 0)).astype(NP_BF16)
    sinK = np.ascontiguousarray(np.concatenate([-s64, s64], 0)).astype(NP_BF16)

    # V tables in row layout [p, m, d<64]; storage row j = m*128+p.
    cosV = np.empty((128, 16, 64), np.float32)
    sinV = np.empty((128, 16, 64), np.float32)
    for m in range(16):
        tj = PERM_K[m * 128 + np.arange(128)]
        cosV[:, m, :] = cos[tj, :]
        sinV[:, m, :] = sin[tj, :]
    return cosK, sinK, \
        np.ascontiguousarray(cosV.reshape(128, 1024)).astype(NP_BF16), \
        np.ascontiguousarray(sinV.reshape(128, 1024)).astype(NP_BF16)


def _build_nc(with_bias_qk: bool):
    nc = bacc.Bacc(None)
    dp = nc.declare_dram_parameter
    qT = dp("qT", [128, 256], F32R, isOutput=False)
    kT = dp("kT", [128, 256], F32R, isOutput=False)
    vT = dp("vT", [128, 256], F32R, isOutput=False)
    wq = dp("wq", [128, H * D], F32R, isOutput=False)      # x4 folded on host
    wk = dp("wk", [128, KVH * D], F32R, isOutput=False)
    wv = dp("wv", [128, KVH * D], F32R, isOutput=False)
    wob = dp("wob", [128, 4 * 128], BF16, isOutput=False)  # [d, h, dout]
    bq = dp("bq", [128, 32], F32, isOutput=False)          # x4 folded on host
    bk = dp("bk", [128, 8], F32, isOutput=False)
    bv = dp("bv", [1, KVH * D], F32R, isOutput=False)
    cosK = dp("cosK", [128, S], BF16, isOutput=False)
    sinK = dp("sinK", [128, S], BF16, isOutput=False)
    cosV = dp("cosV", [128, 1024], BF16, isOutput=False)
    sinV = dp("sinV", [128, 1024], BF16, isOutput=False)
    onesr = dp("onesr", [1, 128], F32R, isOutput=False)
    ones8 = dp("ones8", [128, 256], FP8, isOutput=False)   # [128,(2,128)] of 1
    out = dp("out", [128, S], F32, isOutput=True)

    ADD = mybir.AluOpType.add
    SUB = mybir.AluOpType.subtract
    MUL = mybir.AluOpType.mult
    EXP = mybir.ActivationFunctionType.Exp
    COPY = mybir.ActivationFunctionType.Copy

    with tile.TileContext(nc) as tc:
        with tc.tile_pool(name="cst", bufs=1) as cst, \
             tc.tile_pool(name="big", bufs=1) as big, \
             tc.tile_pool(name="prp", bufs=2) as prp, \
             tc.tile_pool(name="pr8p", bufs=2) as pr8p, \
             tc.tile_pool(name="rcpool", bufs=2) as rcpool, \
             tc.tile_pool(name="scp", bufs=2, space="PSUM") as scp, \
             tc.tile_pool(name="avp", bufs=1, space="PSUM") as avp, \
             tc.tile_pool(name="dnp", bufs=1, space="PSUM") as dnp:
            # ---- constant / input tiles ----
            qT_sb = cst.tile([128, 256], F32R, tag="qT")
            kT_sb = cst.tile([128, 256], F32R, tag="kT")
            vT_sb = cst.tile([128, 256], F32R, tag="vT")
            wq_sb = cst.tile([128, H * D], F32R, tag="wq")
            wk_sb = cst.tile([128, KVH * D], F32R, tag="wk")
            wv_sb = cst.tile([128, KVH * D], F32R, tag="wv")
            wob_sb = cst.tile([128, 4 * 128], BF16, tag="wob")
            bq_sb = cst.tile([128, 32], F32, tag="bq")
            bk_sb = cst.tile([128, 8], F32, tag="bk")
            bv_sb = cst.tile([1, KVH * D], F32R, tag="bv")
            cosK_sb = cst.tile([128, S], BF16, tag="cosK")
            sinK_sb = cst.tile([128, S], BF16, tag="sinK")
            cosV_sb = cst.tile([128, 1024], BF16, tag="cosV")
            sinV_sb = cst.tile([128, 1024], BF16, tag="sinV")
            onesr_sb = cst.tile([1, 128], F32R, tag="onesr")
            ones8_sb = cst.tile([128, 256], FP8, tag="ones8")

            # DMA distribution: sync = K-side + Wq lo, scalar (ACT queue,
            # idle before the first exp) = V-side + Wq hi, gpsimd = rest.
            nc.sync.dma_start(out=kT_sb[:], in_=kT[:])
            nc.sync.dma_start(out=wk_sb[:], in_=wk[:])
            nc.sync.dma_start(out=wq_sb[:, 0:1024], in_=wq[:, 0:1024])
            nc.sync.dma_start(out=wq_sb[:, 1024:2048], in_=wq[:, 1024:2048])
            nc.sync.dma_start(out=wq_sb[:, 2048:3072], in_=wq[:, 2048:3072])
            nc.sync.dma_start(out=wq_sb[:, 3072:4096], in_=wq[:, 3072:4096])
            nc.scalar.dma_start(out=vT_sb[:], in_=vT[:])
            nc.scalar.dma_start(out=wv_sb[:], in_=wv[:])
            nc.gpsimd.dma_start(out=qT_sb[:], in_=qT[:])
            nc.gpsimd.dma_start(out=bv_sb[:], in_=bv[:])
            nc.gpsimd.dma_start(out=onesr_sb[:], in_=onesr[:])
            nc.gpsimd.dma_start(out=cosK_sb[:], in_=cosK[:])
            nc.gpsimd.dma_start(out=sinK_sb[:], in_=sinK[:])
            nc.gpsimd.dma_start(out=cosV_sb[:], in_=cosV[:])
            nc.gpsimd.dma_start(out=sinV_sb[:], in_=sinV[:])
            nc.gpsimd.dma_start(out=ones8_sb[:], in_=ones8[:])
            nc.gpsimd.dma_start(out=wob_sb[:], in_=wob[:])
            nc.gpsimd.dma_start(out=bk_sb[:], in_=bk[:])
            nc.gpsimd.dma_start(out=bq_sb[:], in_=bq[:])

            # ---- working tensors ----
            KTpre = big.tile([128, S], BF16, tag="KTpre")
            KTrot = big.tile([128, S], BF16, tag="KTrot")
            KR = big.tile([128, S], BF16, tag="KR")
            kT8 = big.tile([64, 2, S], FP8, tag="kT8")
            t1k = big.tile([128, S], BF16, tag="t1k")
            VRpre = big.tile([128, S], BF16, tag="VRpre")   # [p,(m,d)]
            vrb = big.tile([128, 16, 128], BF16, tag="vrb")
            tva = big.tile([128, 16, 64], BF16, tag="tva")
            tvb = big.tile([128, 16, 64], BF16, tag="tvb")
            tvc = big.tile([128, 16, 64], BF16, tag="tvc")
            tvd = big.tile([128, 16, 64], BF16, tag="tvd")
            qT8 = big.tile([64, 2 * 4 * 2048], FP8, tag="qT8")  # (i,h,b,a)
            OHT = big.tile([128, 4, 4, 512], BF16, tag="OHT")   # (d,h,c,jq)
            out_sb = big.tile([128, S], F32, tag="osb")

            ones8v = ones8_sb[:].rearrange("p (i d) -> p i d", i=2)
            wobv = wob_sb[:].rearrange("p (h d) -> p h d", h=4)
            cosVv = cosV_sb[:].rearrange("p (m d) -> p m d", m=16)
            sinVv = sinV_sb[:].rearrange("p (m d) -> p m d", m=16)
            v4 = VRpre[:].rearrange("p (bb two d) -> p bb two d", bb=8, two=2)
            v4a = VRpre[:].rearrange("p (bb two d) -> p two bb d", bb=8, two=2)
            v3 = VRpre[:].rearrange("p (m d) -> p m d", m=16)
            qT8i = qT8[:].rearrange("p (i x) -> p i x", i=2)      # x=(h,b,a)
            qT8g = qT8[:].rearrange("p (i h b a) -> p i h b a", i=2, h=4, b=32)

            # ---- projections (PE), each tile drained right away ----
            kdr = []
            for t_ in range(2):
                pk = scp.tile([128, 1536], F32, tag="sc", name=f"pk{t_}")
                for j in range(4):
                    b = 4 * t_ + j
                    nc.tensor.matmul(pk[:, j * 256:(j + 1) * 256],
                                     wk_sb[:, b * 128:(b + 1) * 128],
                                     kT_sb[:], start=True, stop=True)
                kdr.append(pk)
                if with_bias_qk:
                    for j in range(4):
                        b = 4 * t_ + j
                        nc.vector.tensor_scalar(
                            KTpre[:, b * 256:(b + 1) * 256],
                            pk[:, j * 256:(j + 1) * 256],
                            bk_sb[:, b:b + 1], None, ADD)
                else:
                    nc.vector.tensor_copy(KTpre[:, t_ * 1024:(t_ + 1) * 1024],
                                          pk[:, 0:1024])
                # rotate-half copy (partition swap) via DMA, no engine op
                cs_ = slice(t_ * 1024, (t_ + 1) * 1024)
                nc.sync.dma_start(out=KTrot[0:64, cs_], in_=KTpre[64:128, cs_])
                nc.sync.dma_start(out=KTrot[64:128, cs_], in_=KTpre[0:64, cs_])
            vdr = []
            for bg in range(2):
                pv = scp.tile([128, 1536], F32, tag="sc", name=f"pv{bg}")
                for ah in range(2):
                    sl = slice(ah * 512, (ah + 1) * 512)
                    # stop=True is sim-only group bookkeeping; the second
                    # matmul (skip_group_check) still accumulates on top
                    nc.tensor.matmul(pv[:, sl], onesr_sb[:],
                                     bv_sb[:, bg * 512:(bg + 1) * 512],
                                     start=True, stop=True)
                    nc.tensor.matmul(pv[:, sl],
                                     vT_sb[:, ah * 128:(ah + 1) * 128],
                                     wv_sb[:, bg * 512:(bg + 1) * 512],
                                     start=False, stop=True,
                                     skip_group_check=True)
                vdr.append(pv)
                # one op over both ah halves; flat psum in, rearranged out
                nc.scalar.activation(
                    v4a[:, :, 4 * bg:4 * bg + 4, :],
                    pv[:, 0:1024]
                    .rearrange("p (ah b d) -> p ah b d", ah=2, b=4),
                    COPY)
            qdr = []
            for t_ in range(2):
                pq = scp.tile([128, 1536], F32, tag="sc", name=f"pq{t_}")
                for j in range(4):
                    b = 4 * t_ + j
                    nc.tensor.matmul(pq[:, j * 256:(j + 1) * 256],
                                     wq_sb[:, b * 128:(b + 1) * 128],
                                     qT_sb[:], start=True, stop=True)
                qdr.append(pq)
                src_ = pq[:, 0:1024].rearrange("p (b h a) -> p h b a",
                                               b=4, h=4)
                if with_bias_qk:
                    for j in range(4):
                        b = 4 * t_ + j
                        for i in range(2):
                            nc.vector.tensor_scalar(
                                qT8g[:, i, :, b, :],
                                src_[i * 64:(i + 1) * 64, :, j, :],
                                bq_sb[i * 64:(i + 1) * 64, b:b + 1],
                                None, ADD)
                else:
                    for i in range(2):
                        nc.scalar.activation(
                            qT8g[:, i, :, 4 * t_:4 * t_ + 4, :],
                            src_[i * 64:(i + 1) * 64], COPY)

            def k_rope_slice(s_):
                cs = slice(512 * s_, 512 * (s_ + 1))
                nc.vector.tensor_tensor(t1k[:, cs], KTrot[:, cs],
                                        sinK_sb[:, cs], MUL)
                nc.vector.tensor_tensor(KR[:, cs], KTpre[:, cs],
                                        cosK_sb[:, cs], MUL)
                nc.vector.tensor_tensor(KR[:, cs], KR[:, cs],
                                        t1k[:, cs], ADD)
                # d-split to fp8 via gpsimd cast-DMA (partition remap)
                nc.gpsimd.dma_start(out=kT8[:, 0, cs], in_=KR[0:64, cs])
                nc.gpsimd.dma_start(out=kT8[:, 1, cs], in_=KR[64:128, cs])

            def q_proj_small(pool, b0):
                # in-loop: 2 b-blocks into a [128,512] av/dn-pool psum tile
                pt = pool.tile([128, 512], F32, tag=("av" if pool is avp
                                                     else "dn"),
                               name=f"qp_{b0}")
                for j in range(2):
                    b = b0 + j
                    nc.tensor.matmul(pt[:, j * 256:(j + 1) * 256],
                                     wq_sb[:, b * 128:(b + 1) * 128],
                                     qT_sb[:], start=True, stop=True)
                return pt

            def q_drain_small(pt, b0):
                src_ = pt[:, 0:512].rearrange("p (b h a) -> p h b a",
                                              b=2, h=4)
                if with_bias_qk:
                    for j in range(2):
                        b = b0 + j
                        for i in range(2):
                            nc.vector.tensor_scalar(
                                qT8g[:, i, :, b, :],
                                src_[i * 64:(i + 1) * 64, :, j, :],
                                bq_sb[i * 64:(i + 1) * 64, b:b + 1],
                                None, ADD)
                else:
                    for i in range(2):
                        nc.vector.tensor_copy(
                            qT8g[:, i, :, b0:b0 + 2, :],
                            src_[i * 64:(i + 1) * 64])

            # V rope on Pool (gpsimd), m-groups of 4
            def v_rope_slice(g_):
                ms = slice(4 * g_, 4 * (g_ + 1))
                nc.gpsimd.tensor_tensor(tva[:, ms, :], v3[:, ms, 64:128],
                                        sinVv[:, ms, :], MUL)
                nc.gpsimd.tensor_tensor(tvb[:, ms, :], v3[:, ms, 0:64],
                                        sinVv[:, ms, :], MUL)
                nc.gpsimd.tensor_tensor(tvc[:, ms, :], v3[:, ms, 0:64],
                                        cosVv[:, ms, :], MUL)
                nc.gpsimd.tensor_tensor(tvd[:, ms, :], v3[:, ms, 64:128],
                                        cosVv[:, ms, :], MUL)
                nc.gpsimd.tensor_tensor(vrb[:, ms, 0:64], tvc[:, ms, :],
                                        tva[:, ms, :], SUB)
                nc.gpsimd.tensor_tensor(vrb[:, ms, 64:128], tvd[:, ms, :],
                                        tvb[:, ms, :], ADD)

            # exp act-table preload (ACT, overlapped with fill)
            dume = cst.tile([1, 128], F32, tag="dume")
            nc.scalar.activation(dume[:], onesr_sb[:], EXP, scale=1.0)
            for s_ in range(4):
                k_rope_slice(s_)
            for g_ in range(4):
                v_rope_slice(g_)

            # ---- attention steps (chunk c, head h), software-pipelined ----
            def issue_scores(c, h):
                pr = prp.tile([128, 16, 512], BF16, tag="pr",
                              name=f"pr_{c}_{h}")
                pr8 = pr8p.tile([128, 16, 512], FP8, tag="pr8",
                                name=f"pr8_{c}_{h}")
                rhs = qT8i[:, :, h * 2048 + c * 512:h * 2048 + (c + 1) * 512]
                for (g0, g1) in GROUPS:
                    n = g1 - g0
                    sct = scp.tile([128, 1536], F32, tag="sc",
                                   name=f"sc_{c}_{h}_{g0}")
                    for jk in range(g0, g1):
                        nc.tensor.matmul(
                            sct[:, (jk - g0) * 512:(jk - g0 + 1) * 512],
                            kT8[:, :, jk * 128:(jk + 1) * 128],
                            rhs, start=True, stop=True, perf_mode=DR)
                    nc.scalar.activation(pr[:, g0:g1, :], sct[:, 0:512 * n],
                                         EXP, scale=SCALE / (QK_SCALE ** 2))
                    nc.gpsimd.dma_start(out=pr8[:, g0:g1, :],
                                        in_=pr[:, g0:g1, :])
                return pr, pr8

            def issue_avdn(c, h, pr, pr8):
                av = avp.tile([128, 512], F32, tag="av", name=f"av_{c}_{h}")
                dn = dnp.tile([128, 512], F32, tag="dn", name=f"dn_{c}_{h}")
                for j in range(16):
                    nc.tensor.matmul(av[:], vrb[:, j, :], pr[:, j, :],
                                     start=(j == 0), stop=(j == 15),
                                     skip_group_check=True)
                for j in range(8):
                    nc.tensor.matmul(dn[:], ones8v[:],
                                     pr8[:, 2 * j:2 * j + 2, :],
                                     start=(j == 0), stop=(j == 7),
                                     perf_mode=DR, skip_group_check=True)
                rc = rcpool.tile([128, 512], F32, tag="rc",
                                 name=f"rc_{c}_{h}")
                nc.vector.reciprocal(rc[:], dn[:])
                nc.vector.tensor_tensor(OHT[:, h, c, :], av[:], rc[:], MUL)

            def issue_wo(c):
                po = dnp.tile([128, 512], F32, tag="dn", name=f"po_{c}")
                for h in range(4):
                    nc.tensor.matmul(po[:], wobv[:, h, :], OHT[:, h, c, :],
                                     start=(h == 0), stop=(h == 3),
                                     skip_group_check=True)
                nc.vector.tensor_copy(out_sb[:, c * 512:(c + 1) * 512], po[:])
                nc.sync.dma_start(out=out[:, c * 512:(c + 1) * 512],
                                  in_=out_sb[:, c * 512:(c + 1) * 512])

            steps = [(c, h) for c in range(4) for h in range(4)]
            pend = None
            for (c, h) in steps:
                pr, pr8 = issue_scores(c, h)
                if pend is not None:
                    pc, ph, ppr, ppr8 = pend
                    issue_avdn(pc, ph, ppr, ppr8)
                    if ph == 3:
                        issue_wo(pc)
                if c < 3:
                    # stream 2 b-blocks of chunk c+1's Q projection
                    b0 = 8 * (c + 1) + 2 * h
                    pt = q_proj_small(avp if h % 2 == 0 else dnp, b0)
                    q_drain_small(pt, b0)
                pend = (c, h, pr, pr8)
            pc, ph, ppr, ppr8 = pend
            issue_avdn(pc, ph, ppr, ppr8)
            issue_wo(pc)

    nc.compile()
    return nc


def _get_nc(with_bias_qk: bool = False):
    key = ("nc", with_bias_qk)
    if key not in _nc_cache:
        _nc_cache[key] = _build_nc(with_bias_qk)
    return _nc_cache[key]


def make_in_maps(query, keys, values, Wq, bq, Wk, bk, Wv, bv, Wo, bo):
    cosK, sinK, cosV, sinV = _rope_tables()
    q2 = np.asarray(query, np.float32).reshape(S, D)
    k2 = np.asarray(keys, np.float32).reshape(S, D)
    v2 = np.asarray(values, np.float32).reshape(S, D)
    Wq_ = np.ascontiguousarray(np.asarray(Wq, np.float32) * QK_SCALE)
    Wk_ = np.ascontiguousarray(np.asarray(Wk, np.float32))
    Wv_ = np.ascontiguousarray(np.asarray(Wv, np.float32))
    Wo_ = np.asarray(Wo, np.float32)
    bq_ = np.asarray(bq, np.float32).reshape(32, 128).T.copy() * QK_SCALE
    bk_ = np.asarray(bk, np.float32).reshape(8, 128).T.copy()
    bv_ = np.asarray(bv, np.float32).reshape(1, KVH * D).copy()
    ones_r = np.ones((1, 128), np.float32)
    ones_8 = np.ones((128, 256), NP_FP8)

    with_bias = bool(np.any(bq_) or np.any(bk_))
    in_maps = []
    for c in range(NCORES):
        heads = [c + 8 * r for r in range(REP)]
        qrows = np.concatenate([q2[hh * 64:(hh + 1) * 64] for hh in heads])
        # wob layout [128 d, 4 h, 128 dout]
        wob = np.ascontiguousarray(
            np.stack([Wo_[hh * 128:(hh + 1) * 128] for hh in heads], axis=1)
            .reshape(128, 4 * 128)).astype(NP_BF16)
        in_maps.append({
            "qT": np.ascontiguousarray(qrows.T),
            "kT": np.ascontiguousarray(k2[c * 256:(c + 1) * 256].T),
            "vT": np.ascontiguousarray(v2[c * 256:(c + 1) * 256].T),
            "wq": Wq_, "wk": Wk_, "wv": Wv_,
            "wob": wob,
            "bq": bq_, "bk": bk_, "bv": bv_,
            "cosK": cosK, "sinK": sinK, "cosV": cosV, "sinV": sinV,
            "onesr": ones_r, "ones8": ones_8,
        })
    return in_maps, with_bias


def kernel(query, keys, values, Wq, bq, Wk, bk, Wv, bv, Wo, bo):
    in_maps, with_bias = make_in_maps(query, keys, values, Wq, bq, Wk, bk,
                                      Wv, bv, Wo, bo)
    nc = _get_nc(with_bias)
    res = run_bass_kernel_spmd(nc, in_maps, list(range(NCORES)))
    acc = np.zeros((S, D), np.float64)
    for c in range(NCORES):
        o = np.asarray(res.results[c]["out"], np.float32)  # [dout=128, jq]
        acc += o.T
    final = np.empty((S, D), np.float32)
    final[PERM_Q] = acc.astype(np.float32)
    final += np.asarray(bo, np.float32)
    return final.reshape(B, S, D)


# revision 18
# speedup vs baseline: 1.0335x; 1.0335x over previous
"""GQA kernel for trn2, 8 NeuronCores — v2 (ACT-bound pipeline).

Problem: B=1, S=2048, D=128, H=32, KVH=8, REP=4, rope(theta=1e4) on k AND v,
softmax(q@k^T/sqrt(128)) @ v, out @ Wo + bo.  The reference replicates torch
.view() semantics: (B,S,H*D) -> (B,H,S,D) is a FLAT reinterpretation, so
q-head h is rows [h*64,(h+1)*64) of the projection output reinterpreted as
(2048,128), and kv-head g is rows [g*256,(g+1)*256) of the k/v projections.

Sharding: core c owns kv-head g=c and q-heads {c, c+8, c+16, c+24}.
Storage order: q-position j_q = b*64+a <-> s' = 32a+b; kv j_k = b*256+a <->
t = 8a+b.  Host permutes rope tables into storage order, unpermutes output
rows, and sums per-core partials (Wo is a per-head row-block contraction).

v2 engine plan (per core) — the Activation engine (exp: 1 elem/cycle/
partition over 16.8M probs => ~127us busy) is the pinned bottleneck;
everything else hides under it:
 - scores: fp8e4 DoubleRow matmuls, d contracted as 2x64 partitions
   (q,k scaled x4 into fp8 range; the x4 folds into Wq / K-rope tables)
 - exp: ACT psum->sbuf bf16 probs, jk-tile groups {3,3,3,3,2,2} per
   (chunk, head) step; score psum = 2 x [128,1536] double-buffered
 - AV: bf16 V rows (lhsT) x bf16 probs, psum accumulation over 16 jk tiles
 - dn: gpsimd cast-DMA makes an fp8 copy of probs; fp8 ones x fp8 probs
   DoubleRow matmuls (half-rate rows) accumulate the denominators
 - normalize: DVE reciprocal + tensor_tensor -> bf16 OHT
 - Wo: bf16 matmuls accumulating 4 heads; DVE copy -> f32 out; out DMA
PE ~105us < ACT ~127us; DVE ~50us (K rope + drains + normalize);
Pool/gpsimd does the V rope + the pr8 cast-DMA queue.
"""

import sys

sys.path.insert(0, "/opt/trn_rl_repo")

import numpy as np
import ml_dtypes

import concourse.bass as bass
import concourse.mybir as mybir
import concourse.tile as tile
from concourse import bacc
from concourse.bass_utils import run_bass_kernel_spmd

F32 = mybir.dt.float32
F32R = mybir.dt.float32r
BF16 = mybir.dt.bfloat16
FP8 = mybir.dt.float8e4
DR = mybir.MatmulPerfMode.DoubleRow

B, S, D = 1, 2048, 128
H, KVH, REP = 32, 8, 4
NCORES = 8
SCALE = 1.0 / np.sqrt(128.0)
QK_SCALE = 4.0
ROPE_THETA = 10000.0

NP_BF16 = ml_dtypes.bfloat16
NP_FP8 = np.dtype(mybir.dt.np(FP8))

_j = np.arange(S)
PERM_Q = 32 * (_j % 64) + _j // 64
PERM_K = 8 * (_j % 256) + _j // 256

GROUPS = [(0, 3), (3, 6), (6, 9), (9, 12), (12, 14), (14, 16)]

_nc_cache = {}


def _rope_tables():
    inv_freq = 1.0 / (ROPE_THETA ** (np.arange(0, D, 2, dtype=np.float64) / D))
    ang = np.arange(S, dtype=np.float64)[:, None] * inv_freq  # (S, 64)
    cos = np.cos(ang)
    sin = np.sin(ang)

    # K tables in [d, j_k] layout, 128 partitions: cos duplicated across the
    # two d halves, sin with the rotate-half sign folded ([-sin; +sin]).
    # fp8 x4 pre-scale folded in.
    t = PERM_K
    c64 = cos[t, :].T * QK_SCALE   # (64, S)
    s64 = sin[t, :].T * QK_SCALE
    cosK = np.ascontiguousarray(np.concatenate([c64, c64], 0)).astype(NP_BF16)
    sinK = np.ascontiguousarray(np.concatenate([-s64, s64], 0)).astype(NP_BF16)

    # V tables in row layout [p, m, d<64]; storage row j = m*128+p.
    cosV = np.empty((128, 16, 64), np.float32)
    sinV = np.empty((128, 16, 64), np.float32)
    for m in range(16):
        tj = PERM_K[m * 128 + np.arange(128)]
        cosV[:, m, :] = cos[tj, :]
        sinV[:, m, :] = sin[tj, :]
    return cosK, sinK, \
        np.ascontiguousarray(cosV.reshape(128, 1024)).astype(NP_BF16), \
        np.ascontiguousarray(sinV.reshape(128, 1024)).astype(NP_BF16)


def _build_nc(with_bias_qk: bool):
    nc = bacc.Bacc(None)
    dp = nc.declare_dram_parameter
    qT = dp("qT", [128, 256], F32R, isOutput=False)
    kT = dp("kT", [128, 256], F32R, isOutput=False)
    vT = dp("vT", [128, 256], F32R, isOutput=False)
    wq = dp("wq", [128, H * D], F32R, isOutput=False)      # x4 folded on host
    wk = dp("wk", [128, KVH * D], F32R, isOutput=False)
    wv = dp("wv", [128, KVH * D], F32R, isOutput=False)
    wob = dp("wob", [128, 4 * 128], BF16, isOutput=False)  # [d, h, dout]
    bq = dp("bq", [128, 32], F32, isOutput=False)          # x4 folded on host
    bk = dp("bk", [128, 8], F32, isOutput=False)
    bv = dp("bv", [1, KVH * D], F32R, isOutput=False)
    cosK = dp("cosK", [128, S], BF16, isOutput=False)
    sinK = dp("sinK", [128, S], BF16, isOutput=False)
    cosV = dp("cosV", [128, 1024], BF16, isOutput=False)
    sinV = dp("sinV", [128, 1024], BF16, isOutput=False)
    onesr = dp("onesr", [1, 128], F32R, isOutput=False)
    ones8 = dp("ones8", [128, 256], FP8, isOutput=False)   # [128,(2,128)] of 1
    out = dp("out", [128, S], F32, isOutput=True)

    ADD = mybir.AluOpType.add
    SUB = mybir.AluOpType.subtract
    MUL = mybir.AluOpType.mult
    EXP = mybir.ActivationFunctionType.Exp
    COPY = mybir.ActivationFunctionType.Copy

    with tile.TileContext(nc) as tc:
        with tc.tile_pool(name="cst", bufs=1) as cst, \
             tc.tile_pool(name="big", bufs=1) as big, \
             tc.tile_pool(name="prp", bufs=2) as prp, \
             tc.tile_pool(name="pr8p", bufs=2) as pr8p, \
             tc.tile_pool(name="rcpool", bufs=2) as rcpool, \
             tc.tile_pool(name="scp", bufs=2, space="PSUM") as scp, \
             tc.tile_pool(name="avp", bufs=1, space="PSUM") as avp, \
             tc.tile_pool(name="dnp", bufs=1, space="PSUM") as dnp:
            # ---- constant / input tiles ----
            qT_sb = cst.tile([128, 256], F32R, tag="qT")
            kT_sb = cst.tile([128, 256], F32R, tag="kT")
            vT_sb = cst.tile([128, 256], F32R, tag="vT")
            wq_sb = cst.tile([128, H * D], F32R, tag="wq")
            wk_sb = cst.tile([128, KVH * D], F32R, tag="wk")
            wv_sb = cst.tile([128, KVH * D], F32R, tag="wv")
            wob_sb = cst.tile([128, 4 * 128], BF16, tag="wob")
            bq_sb = cst.tile([128, 32], F32, tag="bq")
            bk_sb = cst.tile([128, 8], F32, tag="bk")
            bv_sb = cst.tile([1, KVH * D], F32R, tag="bv")
            cosK_sb = cst.tile([128, S], BF16, tag="cosK")
            sinK_sb = cst.tile([128, S], BF16, tag="sinK")
            cosV_sb = cst.tile([128, 1024], BF16, tag="cosV")
            sinV_sb = cst.tile([128, 1024], BF16, tag="sinV")
            onesr_sb = cst.tile([1, 128], F32R, tag="onesr")
            ones8_sb = cst.tile([128, 256], FP8, tag="ones8")

            # DMA distribution: sync = K-side + Wq lo, scalar (ACT queue,
            # idle before the first exp) = V-side + Wq hi, gpsimd = rest.
            nc.sync.dma_start(out=kT_sb[:], in_=kT[:])
            nc.sync.dma_start(out=wk_sb[:], in_=wk[:])
            nc.sync.dma_start(out=sinK_sb[:], in_=sinK[:])
            nc.sync.dma_start(out=cosK_sb[:], in_=cosK[:])
            nc.gpsimd.dma_start(out=qT_sb[:], in_=qT[:])
            nc.gpsimd.dma_start(out=vT_sb[:], in_=vT[:])
            nc.gpsimd.dma_start(out=wv_sb[:], in_=wv[:])
            nc.gpsimd.dma_start(out=wq_sb[:, 0:1024], in_=wq[:, 0:1024])
            if with_bias_qk:
                nc.sync.dma_start(out=bv_sb[:], in_=bv[:])
                nc.sync.dma_start(out=onesr_sb[:], in_=onesr[:])
                nc.sync.dma_start(out=bk_sb[:], in_=bk[:])
                nc.sync.dma_start(out=bq_sb[:], in_=bq[:])
            # exp act-table preload (after the scalar-queue DMA issues)
            dume = cst.tile([128, 8], F32, tag="dume")
            nc.scalar.activation(dume[:], kT_sb[:, 0:8], EXP, scale=0.001)


            # ---- working tensors ----
            KTpre = big.tile([128, S], BF16, tag="KTpre")
            KTrot = big.tile([128, S], BF16, tag="KTrot")
            KR = big.tile([128, S], BF16, tag="KR")
            kT8 = big.tile([64, 2, S], FP8, tag="kT8")
            t1k = big.tile([128, S], BF16, tag="t1k")
            VRpre = big.tile([128, S], BF16, tag="VRpre")   # [p,(m,d)]
            vrb = big.tile([128, 16, 128], BF16, tag="vrb")
            tva = big.tile([128, 16, 64], BF16, tag="tva")
            tvb = big.tile([128, 16, 64], BF16, tag="tvb")
            tvc = big.tile([128, 16, 64], BF16, tag="tvc")
            tvd = big.tile([128, 16, 64], BF16, tag="tvd")
            qT8 = big.tile([64, 2 * 4 * 2048], FP8, tag="qT8")  # (i,h,b,a)
            OHT = big.tile([128, 4, 4, 512], BF16, tag="OHT")   # (d,h,c,jq)
            out_sb = big.tile([128, S], F32, tag="osb")

            ones8v = ones8_sb[:].rearrange("p (i d) -> p i d", i=2)
            wobv = wob_sb[:].rearrange("p (h d) -> p h d", h=4)
            cosVv = cosV_sb[:].rearrange("p (m d) -> p m d", m=16)
            sinVv = sinV_sb[:].rearrange("p (m d) -> p m d", m=16)
            v4 = VRpre[:].rearrange("p (bb two d) -> p bb two d", bb=8, two=2)
            v4a = VRpre[:].rearrange("p (bb two d) -> p two bb d", bb=8, two=2)
            v3 = VRpre[:].rearrange("p (m d) -> p m d", m=16)
            qT8i = qT8[:].rearrange("p (i x) -> p i x", i=2)      # x=(h,b,a)
            qT8g = qT8[:].rearrange("p (i h b a) -> p i h b a", i=2, h=4, b=32)

            # ---- projections (PE), each tile drained right away ----
            kdr = []
            for t_ in range(2):
                pk = scp.tile([128, 1536], F32, tag="sc", name=f"pk{t_}")
                for j in range(4):
                    b = 4 * t_ + j
                    nc.tensor.matmul(pk[:, j * 256:(j + 1) * 256],
                                     wk_sb[:, b * 128:(b + 1) * 128],
                                     kT_sb[:], start=True, stop=True)
                kdr.append(pk)
                if with_bias_qk:
                    for j in range(4):
                        b = 4 * t_ + j
                        nc.vector.tensor_scalar(
                            KTpre[:, b * 256:(b + 1) * 256],
                            pk[:, j * 256:(j + 1) * 256],
                            bk_sb[:, b:b + 1], None, ADD)
                else:
                    nc.vector.tensor_copy(KTpre[:, t_ * 1024:(t_ + 1) * 1024],
                                          pk[:, 0:1024])
                # rotate-half copy (partition swap) via DMA, no engine op
                cs_ = slice(t_ * 1024, (t_ + 1) * 1024)
                nc.sync.dma_start(out=KTrot[0:64, cs_], in_=KTpre[64:128, cs_])
                nc.sync.dma_start(out=KTrot[64:128, cs_], in_=KTpre[0:64, cs_])
            qdr = []
            for t_ in range(2):
                pq = scp.tile([128, 1536], F32, tag="sc", name=f"pq{t_}")
                for j in range(4):
                    b = 4 * t_ + j
                    nc.tensor.matmul(pq[:, j * 256:(j + 1) * 256],
                                     wq_sb[:, b * 128:(b + 1) * 128],
                                     qT_sb[:], start=True, stop=True)
                qdr.append(pq)
                src_ = pq[:, 0:1024].rearrange("p (b h a) -> p h b a",
                                               b=4, h=4)
                halves = (0, 1)
                if with_bias_qk:
                    for j in range(4):
                        b = 4 * t_ + j
                        for i in halves:
                            nc.vector.tensor_scalar(
                                qT8g[:, i, :, b, :],
                                src_[i * 64:(i + 1) * 64, :, j, :],
                                bq_sb[i * 64:(i + 1) * 64, b:b + 1],
                                None, ADD)
                else:
                    for i in halves:
                        nc.scalar.activation(
                            qT8g[:, i, :, 4 * t_:4 * t_ + 4, :],
                            src_[i * 64:(i + 1) * 64], COPY)
            vdr = []
            for bg in range(2):
                pv = scp.tile([128, 1536], F32, tag="sc", name=f"pv{bg}")
                for ah in range(2):
                    sl = slice(ah * 512, (ah + 1) * 512)
                    if with_bias_qk:
                        # stop=True is sim-only group bookkeeping; the second
                        # matmul (skip_group_check) still accumulates on top
                        nc.tensor.matmul(pv[:, sl], onesr_sb[:],
                                         bv_sb[:, bg * 512:(bg + 1) * 512],
                                         start=True, stop=True)
                        nc.tensor.matmul(pv[:, sl],
                                         vT_sb[:, ah * 128:(ah + 1) * 128],
                                         wv_sb[:, bg * 512:(bg + 1) * 512],
                                         start=False, stop=True,
                                         skip_group_check=True)
                    else:
                        nc.tensor.matmul(pv[:, sl],
                                         vT_sb[:, ah * 128:(ah + 1) * 128],
                                         wv_sb[:, bg * 512:(bg + 1) * 512],
                                         start=True, stop=True)
                vdr.append(pv)
                # one op over both ah halves; flat psum in, rearranged out
                nc.scalar.activation(
                    v4a[:, :, 4 * bg:4 * bg + 4, :],
                    pv[:, 0:1024]
                    .rearrange("p (ah b d) -> p ah b d", ah=2, b=4),
                    COPY)
            # late constants + wq hi halves (for in-loop Q projections)
            nc.sync.dma_start(out=cosV_sb[:], in_=cosV[:])
            nc.sync.dma_start(out=sinV_sb[:], in_=sinV[:])
            nc.sync.dma_start(out=ones8_sb[:], in_=ones8[:])
            nc.sync.dma_start(out=wob_sb[:], in_=wob[:])
            nc.sync.dma_start(out=wq_sb[:, 1024:2048], in_=wq[:, 1024:2048])
            nc.sync.dma_start(out=wq_sb[:, 2048:3072], in_=wq[:, 2048:3072])
            nc.sync.dma_start(out=wq_sb[:, 3072:4096], in_=wq[:, 3072:4096])

            def k_rope_slice(s_):
                cs = slice(512 * s_, 512 * (s_ + 1))
                # lo half: all inputs on partitions 0-63
                nc.vector.tensor_tensor(t1k[0:64, cs], KTrot[0:64, cs],
                                        sinK_sb[0:64, cs], MUL)
                nc.vector.tensor_tensor(KR[0:64, cs], KTpre[0:64, cs],
                                        cosK_sb[0:64, cs], MUL)
                nc.vector.tensor_tensor(kT8[:, 0, cs], KR[0:64, cs],
                                        t1k[0:64, cs], ADD)
                # hi half: all inputs on partitions 64-127 (out base may differ)
                nc.vector.tensor_tensor(t1k[64:128, cs], KTrot[64:128, cs],
                                        sinK_sb[64:128, cs], MUL)
                nc.vector.tensor_tensor(KR[64:128, cs], KTpre[64:128, cs],
                                        cosK_sb[64:128, cs], MUL)
                nc.vector.tensor_tensor(kT8[:, 1, cs], KR[64:128, cs],
                                        t1k[64:128, cs], ADD)

            def q_proj_small(pool, b0):
                # in-loop: 2 b-blocks into a [128,512] av/dn-pool psum tile
                pt = pool.tile([128, 512], F32, tag=("av" if pool is avp
                                                     else "dn"),
                               name=f"qp_{b0}")
                for j in range(2):
                    b = b0 + j
                    nc.tensor.matmul(pt[:, j * 256:(j + 1) * 256],
                                     wq_sb[:, b * 128:(b + 1) * 128],
                                     qT_sb[:], start=True, stop=True)
                return pt

            def q_drain_small(pt, b0):
                src_ = pt[:, 0:512].rearrange("p (b h a) -> p h b a",
                                              b=2, h=4)
                if with_bias_qk:
                    for j in range(2):
                        b = b0 + j
                        for i in range(2):
                            nc.vector.tensor_scalar(
                                qT8g[:, i, :, b, :],
                                src_[i * 64:(i + 1) * 64, :, j, :],
                                bq_sb[i * 64:(i + 1) * 64, b:b + 1],
                                None, ADD)
                else:
                    for i in range(2):
                        nc.vector.tensor_copy(
                            qT8g[:, i, :, b0:b0 + 2, :],
                            src_[i * 64:(i + 1) * 64])

            # V rope on Pool (gpsimd), m-groups of 4
            def v_rope_slice(g_):
                ms = slice(4 * g_, 4 * (g_ + 1))
                nc.gpsimd.tensor_tensor(tva[:, ms, :], v3[:, ms, 64:128],
                                        sinVv[:, ms, :], MUL)
                nc.gpsimd.tensor_tensor(tvb[:, ms, :], v3[:, ms, 0:64],
                                        sinVv[:, ms, :], MUL)
                nc.gpsimd.tensor_tensor(tvc[:, ms, :], v3[:, ms, 0:64],
                                        cosVv[:, ms, :], MUL)
                nc.gpsimd.tensor_tensor(tvd[:, ms, :], v3[:, ms, 64:128],
                                        cosVv[:, ms, :], MUL)
                nc.gpsimd.tensor_tensor(vrb[:, ms, 0:64], tvc[:, ms, :],
                                        tva[:, ms, :], SUB)
                nc.gpsimd.tensor_tensor(vrb[:, ms, 64:128], tvd[:, ms, :],
                                        tvb[:, ms, :], ADD)

            k_rope_slice(0)
            v_rope_slice(0)
            k_rope_slice(1)
            v_rope_slice(1)
            k_rope_slice(2)
            v_rope_slice(2)
            k_rope_slice(3)
            v_rope_slice(3)

            # ---- attention steps (chunk c, head h), software-pipelined ----
            def issue_scores(c, h):
                pr = prp.tile([128, 16, 512], BF16, tag="pr",
                              name=f"pr_{c}_{h}")
                pr8 = pr8p.tile([128, 16, 512], FP8, tag="pr8",
                                name=f"pr8_{c}_{h}")
                rhs = qT8i[:, :, h * 2048 + c * 512:h * 2048 + (c + 1) * 512]
                for (g0, g1) in GROUPS:
                    n = g1 - g0
                    sct = scp.tile([128, 1536], F32, tag="sc",
                                   name=f"sc_{c}_{h}_{g0}")
                    for jk in range(g0, g1):
                        nc.tensor.matmul(
                            sct[:, (jk - g0) * 512:(jk - g0 + 1) * 512],
                            kT8[:, :, jk * 128:(jk + 1) * 128],
                            rhs, start=True, stop=True, perf_mode=DR)
                    nc.scalar.activation(pr[:, g0:g1, :], sct[:, 0:512 * n],
                                         EXP, scale=SCALE / (QK_SCALE ** 2))
                    nc.gpsimd.dma_start(out=pr8[:, g0:g1, :],
                                        in_=pr[:, g0:g1, :])
                return pr, pr8

            def issue_avdn(c, h, pr, pr8):
                av = avp.tile([128, 512], F32, tag="av", name=f"av_{c}_{h}")
                dn = dnp.tile([128, 512], F32, tag="dn", name=f"dn_{c}_{h}")
                for j in range(16):
                    nc.tensor.matmul(av[:], vrb[:, j, :], pr[:, j, :],
                                     start=(j == 0), stop=(j == 15),
                                     skip_group_check=True)
                for j in range(8):
                    nc.tensor.matmul(dn[:], ones8v[:],
                                     pr8[:, 2 * j:2 * j + 2, :],
                                     start=(j == 0), stop=(j == 7),
                                     perf_mode=DR, skip_group_check=True)
                rc = rcpool.tile([128, 512], F32, tag="rc",
                                 name=f"rc_{c}_{h}")
                nc.vector.reciprocal(rc[:], dn[:])
                nc.vector.tensor_tensor(OHT[:, h, c, :], av[:], rc[:], MUL)

            def issue_wo(c):
                po = dnp.tile([128, 512], F32, tag="dn", name=f"po_{c}")
                for h in range(4):
                    nc.tensor.matmul(po[:], wobv[:, h, :], OHT[:, h, c, :],
                                     start=(h == 0), stop=(h == 3),
                                     skip_group_check=True)
                nc.vector.tensor_copy(out_sb[:, c * 512:(c + 1) * 512], po[:])
                nc.sync.dma_start(out=out[:, c * 512:(c + 1) * 512],
                                  in_=out_sb[:, c * 512:(c + 1) * 512])

            steps = [(c, h) for c in range(4) for h in range(4)]
            pend = None
            for (c, h) in steps:
                pr, pr8 = issue_scores(c, h)
                if pend is not None:
                    pc, ph, ppr, ppr8 = pend
                    issue_avdn(pc, ph, ppr, ppr8)
                    if ph == 3:
                        issue_wo(pc)
                if c < 3:
                    # stream 2 b-blocks of chunk c+1's Q projection
                    b0 = 8 * (c + 1) + 2 * h
                    pt = q_proj_small(avp if h % 2 == 0 else dnp, b0)
                    q_drain_small(pt, b0)
                pend = (c, h, pr, pr8)
            pc, ph, ppr, ppr8 = pend
            issue_avdn(pc, ph, ppr, ppr8)
            issue_wo(pc)

    nc.compile()
    return nc


def _get_nc(with_bias_qk: bool = False):
    key = ("nc", with_bias_qk)
    if key not in _nc_cache:
        _nc_cache[key] = _build_nc(with_bias_qk)
    return _nc_cache[key]


def make_in_maps(query, keys, values, Wq, bq, Wk, bk, Wv, bv, Wo, bo):
    cosK, sinK, cosV, sinV = _rope_tables()
    q2 = np.asarray(query, np.float32).reshape(S, D)
    k2 = np.asarray(keys, np.float32).reshape(S, D)
    v2 = np.asarray(values, np.float32).reshape(S, D)
    Wq_ = np.ascontiguousarray(np.asarray(Wq, np.float32) * QK_SCALE)
    Wk_ = np.ascontiguousarray(np.asarray(Wk, np.float32))
    Wv_ = np.ascontiguousarray(np.asarray(Wv, np.float32))
    Wo_ = np.asarray(Wo, np.float32)
    bq_ = np.asarray(bq, np.float32).reshape(32, 128).T.copy() * QK_SCALE
    bk_ = np.asarray(bk, np.float32).reshape(8, 128).T.copy()
    bv_ = np.asarray(bv, np.float32).reshape(1, KVH * D).copy()
    ones_r = np.ones((1, 128), np.float32)
    ones_8 = np.ones((128, 256), NP_FP8)

    with_bias = bool(np.any(bq_) or np.any(bk_) or np.any(bv_))
    in_maps = []
    for c in range(NCORES):
        heads = [c + 8 * r for r in range(REP)]
        qrows = np.concatenate([q2[hh * 64:(hh + 1) * 64] for hh in heads])
        # wob layout [128 d, 4 h, 128 dout]
        wob = np.ascontiguousarray(
            np.stack([Wo_[hh * 128:(hh + 1) * 128] for hh in heads], axis=1)
            .reshape(128, 4 * 128)).astype(NP_BF16)
        in_maps.append({
            "qT": np.ascontiguousarray(qrows.T),
            "kT": np.ascontiguousarray(k2[c * 256:(c + 1) * 256].T),
            "vT": np.ascontiguousarray(v2[c * 256:(c + 1) * 256].T),
            "wq": Wq_, "wk": Wk_, "wv": Wv_,
            "wob": wob,
            "bq": bq_, "bk": bk_, "bv": bv_,
            "cosK": cosK, "sinK": sinK, "cosV": cosV, "sinV": sinV,
            "onesr": ones_r, "ones8": ones_8,
        })
    return in_maps, with_bias


def kernel(query, keys, values, Wq, bq, Wk, bk, Wv, bv, Wo, bo):
    in_maps, with_bias = make_in_maps(query, keys, values, Wq, bq, Wk, bk,
                                      Wv, bv, Wo, bo)
    nc = _get_nc(with_bias)
    res = run_bass_kernel_spmd(nc, in_maps, list(range(NCORES)))
    acc = np.zeros((S, D), np.float64)
    for c in range(NCORES):
        o = np.asarray(res.results[c]["out"], np.float32)  # [dout=128, jq]
        acc += o.T
    final = np.empty((S, D), np.float32)
    final[PERM_Q] = acc.astype(np.float32)
    final += np.asarray(bo, np.float32)
    return final.reshape(B, S, D)
